# revision 29
# baseline (speedup 1.0000x reference)
"""INT4 MoE grouped-GEMM kernel for Trainium2 (8 NeuronCores), FP8 DoubleRow.

Strategy
--------
Per token t routed to expert e = expert_ids[t]:

    out[t, f] = sum_h inputs[t, h] * W[e, f, h],   W = (q - zp[e,f]) * scale[e,f]

q are the raw int4 nibbles (0..15) - exactly representable in fp8-e4m3.
scale and zero_point fold out of the matmul entirely:

    out = s_f * (x . q) - s_f * zp_f * (sum_h x)

so the device computes only R = x8 . q with BOTH operands fp8-e4m3, using
the PE's DoubleRow perf mode (2 fp8 weights per cell, 256-deep contraction
tiles, 2x bf16 throughput). x8 = e4m3(x) loses ~2.6% RMS per element; the
first-moment part of that error is removed exactly on the host with a
rank-1 correction (qbar_f * D_t, where D = sum of quantization deltas),
leaving measured rel err ~1.5e-2 vs the 2e-2 gate.

Sharding: output-feature parallel. Every core processes ALL tokens but only
a 1024-wide slice of the F=8192 output features (of every expert) -
perfectly load-balanced for any token->expert distribution, no collectives.

Device GEMM: weights stationary, tokens streaming, per-core streaming floor
= T*FT*KT8 columns / 2.4GHz = 218us. v2 closes the gap to that floor:
  * DRAM layouts are DMA-native: weights [E, p, kt, i, FC] (16KB contiguous
    per partition per expert) and x packed per chunk [p][kt][i][n16] (8KB
    per partition per chunk), so every transfer runs at full descriptor
    efficiency instead of 0.5-1KB scatter descriptors.
  * Chunk sizes padded to multiples of 16 (DoubleRow i-step constraint);
    the matmul streams the padded column count, the eviction/out DMA use
    the true count.
  * Expert weights prefetch one expert ahead on the sync+gpsimd queues.
  * Warm-up matmuls read a memset SBUF tile (no DMA dependency), so the
    PE HAM ramp starts right after the runtime prologue instead of after
    the first weight DMA lands.
  * The last expert's outputs evict per 512-chunk and fan out across four
    DMA queues to minimize the post-matmul tail.
Host applies scale/zp/rank-1 corrections, transposes and unpermutes.
"""

import numpy as np
import ml_dtypes

E = 8          # experts
T = 8192       # tokens
H = 2048       # hidden (contraction)
F = 8192       # output features
NCORES = 8
FC = F // NCORES       # 1024 output features per core
KT8 = H // 256         # 8 k-tiles of 256 (DoubleRow pairs)
FT = FC // 128         # 8 f-tiles of 128 per core
CHUNK = 512            # max token chunk (one PSUM bank of fp32)
GROUP = 2              # token chunks processed per wave (PSUM/SBUF budget)
FP8 = ml_dtypes.float8_e4m3   # TRN FP8_EXP4-compatible (max +-240)

_PROGRAM_CACHE: dict[tuple, object] = {}
LAST_RESULT = None  # populated with BassKernelResults for external inspection


def _ceil16(n):
    return -(-n // 16) * 16


def _chunk_layout(counts):
    """Per-expert token chunks in sorted order: list of lists of (t0, n).
    Sizes are balanced (no tiny ragged chunk - an N<50 matmul still pays a
    ~60-cycle pipeline floor)."""
    layout = []
    t0 = 0
    first_nonempty = True
    for e in range(E):
        c = int(counts[e])
        chunks = []
        if c:
            off = 0
            if first_nonempty and c > CHUNK:
                # the first chunk processed runs as a single-chunk wave while
                # weights are still streaming in; make it full-width so the
                # PE's weight consumption rate stays below DMA delivery
                chunks.append((t0, CHUNK))
                off = CHUNK
            first_nonempty = False
            rest = c - off
            if rest:
                k = -(-rest // CHUNK)        # number of chunks
                # all chunks multiples of 16 except the last: the matmul
                # streams ceil16(n) columns (DoubleRow i-step), so per-chunk
                # raggedness is wasted PE time. ceil-based so last <= CHUNK.
                base16 = min(CHUNK, _ceil16(-(-rest // k)))
                for i in range(k - 1):
                    chunks.append((t0 + off, base16))
                    off += base16
                chunks.append((t0 + off, rest - (k - 1) * base16))
                off = c
        layout.append(chunks)
        t0 += c
    return layout


def _build_program(chunk_ns: tuple[tuple[int, ...], ...]):
    """Build the SPMD Bass program. chunk_ns[e] = tuple of chunk sizes for
    expert e (same program runs on all 8 cores)."""
    import concourse.mybir as mybir
    import concourse.tile as tile
    from concourse import bacc
    from concourse.bass import ts

    DR = mybir.MatmulPerfMode.DoubleRow

    # x chunk records: (expert, t0, xoff_bytes, n, n16) in sorted order
    chunks = []
    t0 = 0
    xoff = 0
    for e in range(E):
        for n in chunk_ns[e]:
            n16 = _ceil16(n)
            chunks.append((e, t0, xoff, n, n16))
            t0 += n
            xoff += 16 * n16
    XTOT = xoff

    nc = bacc.Bacc("TRN2", target_bir_lowering=False)
    # x packed per chunk: per partition p the bytes [kt 8][i 2][n16],
    # chunks concatenated; element h of token t is x[t, kt*256 + i*128 + p]
    xg = nc.declare_dram_parameter("xg", [128, XTOT], mybir.dt.float8e4, isOutput=False)
    # weights per expert contiguous per partition: [E][p][kt][i][FC]
    wT = nc.declare_dram_parameter(
        "wT", [E, 128, KT8, 2, FC], mybir.dt.float8e4, isOutput=False
    )
    out = nc.declare_dram_parameter("out", [FC, T], mybir.dt.float16, isOutput=True)

    # flat wave list across experts: (expert, [chunk records]) of <=GROUP
    waves = []
    by_e = {}
    for rec in chunks:
        by_e.setdefault(rec[0], []).append(rec)
    active = [e for e in range(E) if e in by_e]
    first_e = active[0] if active else None
    last_e_pre = active[-1] if active else None
    for e in active:
        recs = by_e[e]
        # the last expert runs single-chunk waves: its per-chunk eviction
        # flushes all but the final chunk during compute, halving the tail
        g = 1 if (e == last_e_pre and len(active) > 1) else GROUP
        if e == first_e and len(recs) > 1:
            # the first wave is a single chunk (the kt-outer ramp)
            waves.append((e, recs[:1]))
            rest = recs[1:]
            waves += [(e, rest[i : i + g]) for i in range(0, len(rest), g)]
        else:
            waves += [(e, recs[i : i + g]) for i in range(0, len(recs), g)]
    next_active = {e: (active[i + 1] if i + 1 < len(active) else None)
                   for i, e in enumerate(active)}
    last_e = active[-1] if active else None

    with tile.TileContext(nc) as tc:
        with (
            tc.tile_pool(name="warm", bufs=1) as warmpool,
            tc.tile_pool(name="wpool", bufs=3) as wpool,
            tc.tile_pool(name="xpool", bufs=6) as xpool,
            tc.tile_pool(name="opool", bufs=12) as opool,
            tc.tile_pool(name="pspool", bufs=8, space="PSUM") as pspool,
        ):
            # ---- queue discipline ----
            # sync+gpsimd: weights (and tail flush); scalar: x chunks;
            # gpsimd: out DMAs; vector: PSUM evictions (casts) only.

            def issue_x(wi, split=False):
                xs = []
                for (_, _, xoff_c, n, n16) in waves[wi][1]:
                    x_c = xpool.tile([128, KT8, 2, n16], mybir.dt.float8e4, name="x_c")
                    src = xg[:, xoff_c : xoff_c + 16 * n16].rearrange(
                        "p (kt i n) -> p kt i n", kt=KT8, i=2
                    )
                    if split:
                        # two halves so the ramp's first matmuls only gate on
                        # the kt0-3 slice
                        nc.scalar.dma_start(
                            out=x_c[:, 0:4, :, :], in_=src[:, 0:4, :, :]
                        )
                        nc.scalar.dma_start(
                            out=x_c[:, 4:8, :, :], in_=src[:, 4:8, :, :]
                        )
                    else:
                        nc.scalar.dma_start(out=x_c[:, :, :, :], in_=src)
                    xs.append(x_c)
                wave_xs[wi] = xs

            def issue_w(e, ramp):
                w_e = wpool.tile([128, KT8, 2, FC], mybir.dt.float8e4, name="w_e")
                if ramp:
                    # 1-kt chunks alternating across two queues: 2x delivery
                    # so the kt-outer ramp never starves
                    for wc in range(KT8):
                        q = nc.sync if (wc & 1) == 0 else nc.gpsimd
                        q.dma_start(
                            out=w_e[:, wc : wc + 1, :, :],
                            in_=wT[e][:, wc : wc + 1, :, :],
                        )
                else:
                    nc.sync.dma_start(
                        out=w_e[:, 0:4, :, :], in_=wT[e][:, 0:4, :, :]
                    )
                    nc.gpsimd.dma_start(
                        out=w_e[:, 4:8, :, :], in_=wT[e][:, 4:8, :, :]
                    )
                return w_e

            # warm-up tile: tiny memset on DVE (~0.25us), no DMA dependency.
            # The PE HAM clock needs ~3us of continuous busy to un-throttle;
            # these matmuls start right after the runtime prologue.
            warm = warmpool.tile([128, 2, 128], mybir.dt.float8e4, name="warm")
            nc.vector.memset(warm[:, :, :], 0.0)
            wz = warm

            wave_xs = {}
            issue_x(0, split=True)
            w_tiles = {first_e: issue_w(first_e, ramp=True)}
            if len(waves) > 1:
                issue_x(1)

            cur_e = None
            w_e = None
            pending_prefetch = None
            for wi, (e, wave) in enumerate(waves):
                if e != cur_e:
                    cur_e = e
                    w_e = w_tiles.pop(e)
                    # prefetch the next expert's weights one expert ahead.
                    # For the first expert, wait one wave: the ramp window is
                    # HBM-saturated by e0's weights + the first x chunks, and
                    # e1 isn't needed until ~35us.
                    ne = next_active[e]
                    if ne is not None:
                        if e == first_e:
                            pending_prefetch = ne
                        else:
                            w_tiles[ne] = issue_w(ne, ramp=False)
                elif pending_prefetch is not None:
                    w_tiles[pending_prefetch] = issue_w(pending_prefetch, ramp=False)
                    pending_prefetch = None

                if wi + 1 < len(waves) and (wi + 1) not in wave_xs:
                    issue_x(wi + 1)
                xs = wave_xs.pop(wi)

                if wi == 0:
                    # ---- ramp wave ----
                    (_, ct0, _, n, n16) = wave[0]
                    pss = [
                        pspool.tile([128, CHUNK], mybir.dt.float32, name="ps")
                        for _ in range(FT)
                    ]
                    # HAM warm-up on the memset tile (no data dependency).
                    # N=128 warmups are LDWEIGHTS-bound (~334ns each) but
                    # keep the PE busy enough for the HAM ramp, and their
                    # tiny memset lets them start ~1.5us earlier.
                    for _ in range(24):
                        nc.tensor.matmul(
                            pss[0][:, :128],
                            lhsT=wz,
                            rhs=wz,
                            start=True,
                            stop=True,
                            perf_mode=DR,
                        )
                    # kt-outer / ft-inner over all 8 PSUM banks: each arriving
                    # kt weight chunk unblocks 8 matmuls (the warm-up garbage
                    # in pss[0] is reset by the start=True group)
                    for kt in range(KT8):
                        for ft in range(FT):
                            nc.tensor.matmul(
                                pss[ft][:, :n16],
                                lhsT=w_e[:, kt, :, ts(ft, 128)],
                                rhs=xs[0][:, kt, :, :],
                                start=(kt == 0),
                                stop=(kt == KT8 - 1),
                                perf_mode=DR,
                            )
                    # ramp evictions burst all at once (every bank stops at
                    # kt==7). All casts on DVE: using scalar.copy anywhere
                    # would insert a ~1.3us ACT_TABLE_LOAD at startup that
                    # delays the scalar queue's first x DMA trigger.
                    for ft in range(FT):
                        o_c = opool.tile([128, CHUNK], mybir.dt.float16, name="o_c")
                        nc.vector.tensor_copy(o_c[:, :n], pss[ft][:, :n])
                        nc.gpsimd.dma_start(
                            out=out[ts(ft, 128), ct0 : ct0 + n], in_=o_c[:, :n]
                        )
                    continue

                for ft in range(FT):
                    pss = [
                        pspool.tile([128, CHUNK], mybir.dt.float32, name="ps")
                        for _ in wave
                    ]
                    for kt in range(KT8):
                        for ci, (_, _, _, n, n16) in enumerate(wave):
                            nc.tensor.matmul(
                                pss[ci][:, :n16],
                                lhsT=w_e[:, kt, :, ts(ft, 128)],
                                rhs=xs[ci][:, kt, :, :],
                                start=(kt == 0),
                                stop=(kt == KT8 - 1),
                                perf_mode=DR,
                            )
                    if e == last_e:
                        # tail: evict per chunk, out DMAs on scalar+sync ONLY.
                        # gpsimd must finish its DMA work well before the end:
                        # the Tile epilogue's gpsimd dge_drain (dma_reset) takes
                        # ~10us and only overlaps compute if gpsimd's stream
                        # retires early.
                        for ci, (_, ct0, _, n, n16) in enumerate(wave):
                            o_c = opool.tile(
                                [128, CHUNK], mybir.dt.float16, name="o_c"
                            )
                            nc.vector.tensor_copy(o_c[:, :n], pss[ci][:, :n])
                            q = (nc.scalar, nc.sync)[(2 * ft + ci) % 2]
                            q.dma_start(
                                out=out[ts(ft, 128), ct0 : ct0 + n],
                                in_=o_c[:, :n],
                            )
                    else:
                        # coalesce the wave's eviction into one contiguous
                        # SBUF tile and a single out DMA (chunks are adjacent
                        # token ranges)
                        o_c = opool.tile(
                            [128, GROUP * CHUNK], mybir.dt.float16, name="o_c"
                        )
                        off = 0
                        for ci, (_, _, _, n, n16) in enumerate(wave):
                            nc.vector.tensor_copy(
                                o_c[:, off : off + n], pss[ci][:, :n]
                            )
                            off += n
                        wt0 = wave[0][1]
                        nc.gpsimd.dma_start(
                            out=out[ts(ft, 128), wt0 : wt0 + off], in_=o_c[:, :off]
                        )
    if not nc.is_finalized():
        nc.finalize()
    return nc


def kernel(
    packed_weights: np.ndarray,
    scales: np.ndarray,
    zero_points: np.ndarray,
    inputs: np.ndarray,
    expert_ids: np.ndarray,
    tokens_per_expert: np.ndarray,
    input_offsets: np.ndarray,
) -> np.ndarray:
    global LAST_RESULT
    from concourse.bass_utils import run_bass_kernel_spmd

    packed_weights = np.asarray(packed_weights)
    scales = np.asarray(scales, dtype=np.float32)
    zero_points = np.asarray(zero_points, dtype=np.float32)
    inputs = np.asarray(inputs, dtype=np.float32)
    expert_ids = np.asarray(expert_ids)

    # ---- host routing: sort tokens by expert (robust to unsorted input) ----
    perm = np.argsort(expert_ids, kind="stable")  # sorted order -> orig index
    counts = np.bincount(expert_ids, minlength=E).astype(np.int64)
    layout = _chunk_layout(counts)
    chunk_ns = tuple(tuple(n for _, n in chunks) for chunks in layout)

    # ---- host prep: x sorted, quantized to e4m3 ----
    x_sorted = inputs[perm]                      # [T, H] fp32
    x8_sorted = x_sorted.astype(FP8)             # [T, H] e4m3

    # rank-1 correction ingredients (exact, fp32)
    SX = x_sorted.sum(axis=1, dtype=np.float32)                       # [T]
    D = x8_sorted.astype(np.float32).sum(axis=1, dtype=np.float32) - SX  # [T]

    # pack x per chunk: [p][kt][i][n16] per partition, chunks concatenated
    Y = np.ascontiguousarray(
        x8_sorted.reshape(T, KT8, 2, 128).transpose(3, 1, 2, 0)
    )  # [128, kt, i, T]
    all_chunks = [c for chunks in layout for c in chunks]
    XTOT = 16 * sum(_ceil16(n) for _, n in all_chunks)
    xg_host = np.zeros((128, XTOT), dtype=FP8)
    off = 0
    for (t0, n) in all_chunks:
        n16 = _ceil16(n)
        blk = np.zeros((128, KT8, 2, n16), dtype=FP8)
        blk[:, :, :, :n] = Y[:, :, :, t0 : t0 + n]
        xg_host[:, off : off + 16 * n16] = blk.reshape(128, 16 * n16)
        off += 16 * n16

    # ---- host: raw int4 nibbles -> e4m3 (exact), [E, H, F] ----
    b = (packed_weights & 0xFF).astype(np.uint8)      # [E, F, P] byte values
    lo = (b & 0xF)                                    # even h = 2p
    hi = (b >> 4)                                     # odd  h = 2p+1
    W8 = np.empty((E, H, F), dtype=np.uint8)
    W8[:, 0::2, :] = lo.transpose(0, 2, 1)
    W8[:, 1::2, :] = hi.transpose(0, 2, 1)
    qbar = (
        lo.sum(axis=2, dtype=np.uint32) + hi.sum(axis=2, dtype=np.uint32)
    ).astype(np.float32) / np.float32(H)              # [E, F] mean_h q
    W8 = W8.astype(FP8)                               # values 0..15: exact

    # ---- build / fetch program ----
    nc = _PROGRAM_CACHE.get(chunk_ns)
    if nc is None:
        nc = _build_program(chunk_ns)
        _PROGRAM_CACHE[chunk_ns] = nc

    in_maps = []
    for c in range(NCORES):
        # [E, H, FC] -> [E, p, kt, i, FC]: h = kt*256 + i*128 + p
        wc = (
            W8[:, :, c * FC : (c + 1) * FC]
            .reshape(E, KT8, 2, 128, FC)
            .transpose(0, 3, 1, 2, 4)
        )
        in_maps.append({"xg": xg_host, "wT": np.ascontiguousarray(wc)})

    res = run_bass_kernel_spmd(nc, in_maps, list(range(NCORES)))
    LAST_RESULT = res

    # ---- gather: stack F-major slices, transpose to sorted [T, F] ----
    R_T = np.concatenate(
        [np.asarray(res.results[c]["out"]) for c in range(NCORES)], axis=0
    )  # [F, T] fp16
    R_sorted = np.ascontiguousarray(R_T.T).astype(np.float32)  # [T, F]

    # ---- host epilogue: fold scale/zp + rank-1 e4m3 mean correction ----
    #   out = s_f * R - s_f*zp_f * SX_t - s_f*qbar_f * D_t
    out_sorted = np.empty((T, F), dtype=np.float32)
    t0 = 0
    for e in range(E):
        c = int(counts[e])
        if c == 0:
            continue
        sl = slice(t0, t0 + c)
        out_sorted[sl] = (
            scales[e][None, :] * R_sorted[sl]
            - np.outer(SX[sl], scales[e] * zero_points[e])
            - np.outer(D[sl], scales[e] * qbar[e])
        )
        t0 += c

    out_full = np.empty((T, F), dtype=np.float32)
    out_full[perm] = out_sorted
    return out_full


# revision 31
# speedup vs baseline: 1.0386x; 1.0386x over previous
"""INT4 MoE grouped-GEMM kernel for Trainium2 (8 NeuronCores), FP8 DoubleRow.

Strategy
--------
Per token t routed to expert e = expert_ids[t]:

    out[t, f] = sum_h inputs[t, h] * W[e, f, h],   W = (q - zp[e,f]) * scale[e,f]

q are the raw int4 nibbles (0..15) - exactly representable in fp8-e4m3.
scale and zero_point fold out of the matmul entirely:

    out = s_f * (x . q) - s_f * zp_f * (sum_h x)

so the device computes only R = x8 . q with BOTH operands fp8-e4m3, using
the PE's DoubleRow perf mode (2 fp8 weights per cell, 256-deep contraction
tiles, 2x bf16 throughput). x8 = e4m3(x) loses ~2.6% RMS per element; the
first-moment part of that error is removed exactly on the host with a
rank-1 correction (qbar_f * D_t, where D = sum of quantization deltas),
leaving measured rel err ~1.5e-2 vs the 2e-2 gate.

Sharding: output-feature parallel. Every core processes ALL tokens but only
a 1024-wide slice of the F=8192 output features (of every expert) -
perfectly load-balanced for any token->expert distribution, no collectives.

Device GEMM: weights stationary, tokens streaming, per-core streaming floor
= T*FT*KT8 columns / 2.4GHz = 218us. v2 closes the gap to that floor:
  * DRAM layouts are DMA-native: weights [E, p, kt, i, FC] (16KB contiguous
    per partition per expert) and x packed per chunk [p][kt][i][n16] (8KB
    per partition per chunk), so every transfer runs at full descriptor
    efficiency instead of 0.5-1KB scatter descriptors.
  * Chunk sizes padded to multiples of 16 (DoubleRow i-step constraint);
    the matmul streams the padded column count, the eviction/out DMA use
    the true count.
  * Expert weights prefetch one expert ahead on the sync+gpsimd queues.
  * Warm-up matmuls read a memset SBUF tile (no DMA dependency), so the
    PE HAM ramp starts right after the runtime prologue instead of after
    the first weight DMA lands.
  * The last expert's outputs evict per 512-chunk and fan out across four
    DMA queues to minimize the post-matmul tail.
Host applies scale/zp/rank-1 corrections, transposes and unpermutes.
"""

import numpy as np
import ml_dtypes

E = 8          # experts
T = 8192       # tokens
H = 2048       # hidden (contraction)
F = 8192       # output features
NCORES = 8
FC = F // NCORES       # 1024 output features per core
KT8 = H // 256         # 8 k-tiles of 256 (DoubleRow pairs)
FT = FC // 128         # 8 f-tiles of 128 per core
CHUNK = 512            # max token chunk (one PSUM bank of fp32)
GROUP = 2              # token chunks processed per wave (PSUM/SBUF budget)
FP8 = ml_dtypes.float8_e4m3   # TRN FP8_EXP4-compatible (max +-240)

_PROGRAM_CACHE: dict[tuple, object] = {}
LAST_RESULT = None  # populated with BassKernelResults for external inspection


def _ceil16(n):
    return -(-n // 16) * 16


def _chunk_layout(counts):
    """Per-expert token chunks in sorted order: list of lists of (t0, n).
    Sizes are balanced (no tiny ragged chunk - an N<50 matmul still pays a
    ~60-cycle pipeline floor)."""
    layout = []
    t0 = 0
    first_nonempty = True
    for e in range(E):
        c = int(counts[e])
        chunks = []
        if c:
            off = 0
            if first_nonempty and c > CHUNK:
                # the first chunk processed runs as a single-chunk wave while
                # weights are still streaming in; make it full-width so the
                # PE's weight consumption rate stays below DMA delivery
                chunks.append((t0, CHUNK))
                off = CHUNK
            first_nonempty = False
            rest = c - off
            if rest:
                k = -(-rest // CHUNK)        # number of chunks
                # all chunks multiples of 16 except the last: the matmul
                # streams ceil16(n) columns (DoubleRow i-step), so per-chunk
                # raggedness is wasted PE time. ceil-based so last <= CHUNK.
                base16 = min(CHUNK, _ceil16(-(-rest // k)))
                for i in range(k - 1):
                    chunks.append((t0 + off, base16))
                    off += base16
                chunks.append((t0 + off, rest - (k - 1) * base16))
                off = c
        layout.append(chunks)
        t0 += c
    return layout


def _build_program(chunk_ns: tuple[tuple[int, ...], ...]):
    """Build the SPMD Bass program. chunk_ns[e] = tuple of chunk sizes for
    expert e (same program runs on all 8 cores)."""
    import concourse.mybir as mybir
    import concourse.tile as tile
    from concourse import bacc
    from concourse.bass import ts

    DR = mybir.MatmulPerfMode.DoubleRow

    # x chunk records: (expert, t0, xoff_bytes, n, n16) in sorted order
    chunks = []
    t0 = 0
    xoff = 0
    for e in range(E):
        for n in chunk_ns[e]:
            n16 = _ceil16(n)
            chunks.append((e, t0, xoff, n, n16))
            t0 += n
            xoff += 16 * n16
    XTOT = xoff

    nc = bacc.Bacc("TRN2", target_bir_lowering=False)
    # x packed per chunk: per partition p the bytes [kt 8][i 2][n16],
    # chunks concatenated; element h of token t is x[t, kt*256 + i*128 + p]
    xg = nc.declare_dram_parameter("xg", [128, XTOT], mybir.dt.float8e4, isOutput=False)
    # weights per expert contiguous per partition: [E][p][kt][i][FC]
    wT = nc.declare_dram_parameter(
        "wT", [E, 128, KT8, 2, FC], mybir.dt.float8e4, isOutput=False
    )
    out = nc.declare_dram_parameter("out", [FC, T], mybir.dt.float16, isOutput=True)

    # flat wave list across experts: (expert, [chunk records]) of <=GROUP
    waves = []
    by_e = {}
    for rec in chunks:
        by_e.setdefault(rec[0], []).append(rec)
    active = [e for e in range(E) if e in by_e]
    first_e = active[0] if active else None
    last_e_pre = active[-1] if active else None
    for e in active:
        recs = by_e[e]
        # the last expert runs single-chunk waves: its per-chunk eviction
        # flushes all but the final chunk during compute, halving the tail
        g = 1 if (e == last_e_pre and len(active) > 1) else GROUP
        if e == first_e and len(recs) > 1:
            # the first wave is a single chunk (the kt-outer ramp)
            waves.append((e, recs[:1]))
            rest = recs[1:]
            waves += [(e, rest[i : i + g]) for i in range(0, len(rest), g)]
        else:
            waves += [(e, recs[i : i + g]) for i in range(0, len(recs), g)]
    next_active = {e: (active[i + 1] if i + 1 < len(active) else None)
                   for i, e in enumerate(active)}
    last_e = active[-1] if active else None

    with tile.TileContext(nc) as tc:
        with (
            tc.tile_pool(name="warm", bufs=1) as warmpool,
            tc.tile_pool(name="wpool", bufs=3) as wpool,
            tc.tile_pool(name="xpool", bufs=6) as xpool,
            tc.tile_pool(name="opool", bufs=12) as opool,
            tc.tile_pool(name="pspool", bufs=8, space="PSUM") as pspool,
        ):
            # ---- queue discipline ----
            # sync+gpsimd: weights (and tail flush); scalar: x chunks;
            # gpsimd: out DMAs; vector: PSUM evictions (casts) only.

            def issue_x(wi, split=False):
                xs = []
                for (_, _, xoff_c, n, n16) in waves[wi][1]:
                    x_c = xpool.tile([128, KT8, 2, n16], mybir.dt.float8e4, name="x_c")
                    src = xg[:, xoff_c : xoff_c + 16 * n16].rearrange(
                        "p (kt i n) -> p kt i n", kt=KT8, i=2
                    )
                    if split:
                        # two halves so the ramp's first matmuls only gate on
                        # the kt0-3 slice
                        nc.scalar.dma_start(
                            out=x_c[:, 0:4, :, :], in_=src[:, 0:4, :, :]
                        )
                        nc.scalar.dma_start(
                            out=x_c[:, 4:8, :, :], in_=src[:, 4:8, :, :]
                        )
                    else:
                        nc.scalar.dma_start(out=x_c[:, :, :, :], in_=src)
                    xs.append(x_c)
                wave_xs[wi] = xs

            def issue_w(e, ramp):
                w_e = wpool.tile([128, KT8, 2, FC], mybir.dt.float8e4, name="w_e")
                if ramp:
                    # 1-kt chunks alternating across two queues: 2x delivery
                    # so the kt-outer ramp never starves
                    for wc in range(KT8):
                        q = nc.sync if (wc & 1) == 0 else nc.gpsimd
                        q.dma_start(
                            out=w_e[:, wc : wc + 1, :, :],
                            in_=wT[e][:, wc : wc + 1, :, :],
                        )
                else:
                    nc.sync.dma_start(
                        out=w_e[:, 0:4, :, :], in_=wT[e][:, 0:4, :, :]
                    )
                    nc.gpsimd.dma_start(
                        out=w_e[:, 4:8, :, :], in_=wT[e][:, 4:8, :, :]
                    )
                return w_e

            # warm-up tile: memset on DVE in two pieces, no DMA dependency.
            # The PE HAM clock needs ~3us of continuous busy to un-throttle;
            # the first (tiny) memset lets warm-up matmuls start ~0.7us
            # earlier; the full-width slice keeps the PE at full duty after.
            warm = warmpool.tile([128, 2, 640], mybir.dt.float8e4, name="warm")
            nc.vector.memset(warm[:, :, 0:128], 0.0)
            nc.vector.memset(warm[:, :, 128:640], 0.0)
            wz = warm[:, :, 0:128]
            xz = warm[:, :, 128:640]

            wave_xs = {}
            issue_x(0, split=True)
            w_tiles = {first_e: issue_w(first_e, ramp=True)}
            if len(waves) > 1:
                issue_x(1)

            cur_e = None
            w_e = None
            pending_prefetch = None
            for wi, (e, wave) in enumerate(waves):
                if e != cur_e:
                    cur_e = e
                    w_e = w_tiles.pop(e)
                    # prefetch the next expert's weights one expert ahead.
                    # For the first expert, wait one wave: the ramp window is
                    # HBM-saturated by e0's weights + the first x chunks, and
                    # e1 isn't needed until ~35us.
                    ne = next_active[e]
                    if ne is not None:
                        if e == first_e:
                            pending_prefetch = ne
                        else:
                            w_tiles[ne] = issue_w(ne, ramp=False)
                elif pending_prefetch is not None:
                    w_tiles[pending_prefetch] = issue_w(pending_prefetch, ramp=False)
                    pending_prefetch = None

                if wi + 1 < len(waves) and (wi + 1) not in wave_xs:
                    issue_x(wi + 1)
                xs = wave_xs.pop(wi)

                if wi == 0:
                    # ---- ramp wave ----
                    (_, ct0, _, n, n16) = wave[0]
                    pss = [
                        pspool.tile([128, CHUNK], mybir.dt.float32, name="ps")
                        for _ in range(FT)
                    ]
                    # HAM warm-up on the memset tile (no data dependency):
                    # a few N=128 starters gated only by the tiny memset,
                    # then full-duty N=512 warmups (N=128 alone is
                    # LDWEIGHTS-bound at ~32% PE duty and lets the HAM ramp
                    # slip on unlucky cores)
                    for _ in range(4):
                        nc.tensor.matmul(
                            pss[0][:, :128],
                            lhsT=wz,
                            rhs=wz,
                            start=True,
                            stop=True,
                            perf_mode=DR,
                        )
                    for _ in range(14):
                        nc.tensor.matmul(
                            pss[0][:, :CHUNK],
                            lhsT=wz,
                            rhs=xz,
                            start=True,
                            stop=True,
                            perf_mode=DR,
                        )
                    # kt-outer / ft-inner over all 8 PSUM banks: each arriving
                    # kt weight chunk unblocks 8 matmuls (the warm-up garbage
                    # in pss[0] is reset by the start=True group)
                    for kt in range(KT8):
                        for ft in range(FT):
                            nc.tensor.matmul(
                                pss[ft][:, :n16],
                                lhsT=w_e[:, kt, :, ts(ft, 128)],
                                rhs=xs[0][:, kt, :, :],
                                start=(kt == 0),
                                stop=(kt == KT8 - 1),
                                perf_mode=DR,
                            )
                    # ramp evictions burst all at once (every bank stops at
                    # kt==7). All casts on DVE: using scalar.copy anywhere
                    # would insert a ~1.3us ACT_TABLE_LOAD at startup that
                    # delays the scalar queue's first x DMA trigger.
                    for ft in range(FT):
                        o_c = opool.tile([128, CHUNK], mybir.dt.float16, name="o_c")
                        nc.vector.tensor_copy(o_c[:, :n], pss[ft][:, :n])
                        nc.gpsimd.dma_start(
                            out=out[ts(ft, 128), ct0 : ct0 + n], in_=o_c[:, :n]
                        )
                    continue

                for ft in range(FT):
                    pss = [
                        pspool.tile([128, CHUNK], mybir.dt.float32, name="ps")
                        for _ in wave
                    ]
                    for kt in range(KT8):
                        for ci, (_, _, _, n, n16) in enumerate(wave):
                            nc.tensor.matmul(
                                pss[ci][:, :n16],
                                lhsT=w_e[:, kt, :, ts(ft, 128)],
                                rhs=xs[ci][:, kt, :, :],
                                start=(kt == 0),
                                stop=(kt == KT8 - 1),
                                perf_mode=DR,
                            )
                    if e == last_e:
                        # tail: evict per chunk, out DMAs on scalar+sync ONLY.
                        # gpsimd must finish its DMA work well before the end:
                        # the Tile epilogue's gpsimd dge_drain (dma_reset) takes
                        # ~10us and only overlaps compute if gpsimd's stream
                        # retires early.
                        for ci, (_, ct0, _, n, n16) in enumerate(wave):
                            o_c = opool.tile(
                                [128, CHUNK], mybir.dt.float16, name="o_c"
                            )
                            nc.vector.tensor_copy(o_c[:, :n], pss[ci][:, :n])
                            q = (nc.scalar, nc.sync)[(2 * ft + ci) % 2]
                            q.dma_start(
                                out=out[ts(ft, 128), ct0 : ct0 + n],
                                in_=o_c[:, :n],
                            )
                    else:
                        # coalesce the wave's eviction into one contiguous
                        # SBUF tile and a single out DMA (chunks are adjacent
                        # token ranges)
                        o_c = opool.tile(
                            [128, GROUP * CHUNK], mybir.dt.float16, name="o_c"
                        )
                        off = 0
                        for ci, (_, _, _, n, n16) in enumerate(wave):
                            nc.vector.tensor_copy(
                                o_c[:, off : off + n], pss[ci][:, :n]
                            )
                            off += n
                        wt0 = wave[0][1]
                        nc.gpsimd.dma_start(
                            out=out[ts(ft, 128), wt0 : wt0 + off], in_=o_c[:, :off]
                        )
    if not nc.is_finalized():
        nc.finalize()
    return nc


def kernel(
    packed_weights: np.ndarray,
    scales: np.ndarray,
    zero_points: np.ndarray,
    inputs: np.ndarray,
    expert_ids: np.ndarray,
    tokens_per_expert: np.ndarray,
    input_offsets: np.ndarray,
) -> np.ndarray:
    global LAST_RESULT
    from concourse.bass_utils import run_bass_kernel_spmd

    packed_weights = np.asarray(packed_weights)
    scales = np.asarray(scales, dtype=np.float32)
    zero_points = np.asarray(zero_points, dtype=np.float32)
    inputs = np.asarray(inputs, dtype=np.float32)
    expert_ids = np.asarray(expert_ids)

    # ---- host routing: sort tokens by expert (robust to unsorted input) ----
    perm = np.argsort(expert_ids, kind="stable")  # sorted order -> orig index
    counts = np.bincount(expert_ids, minlength=E).astype(np.int64)
    layout = _chunk_layout(counts)
    chunk_ns = tuple(tuple(n for _, n in chunks) for chunks in layout)

    # ---- host prep: x sorted, quantized to e4m3 ----
    x_sorted = inputs[perm]                      # [T, H] fp32
    x8_sorted = x_sorted.astype(FP8)             # [T, H] e4m3

    # rank-1 correction ingredients (exact, fp32)
    SX = x_sorted.sum(axis=1, dtype=np.float32)                       # [T]
    D = x8_sorted.astype(np.float32).sum(axis=1, dtype=np.float32) - SX  # [T]

    # pack x per chunk: [p][kt][i][n16] per partition, chunks concatenated
    Y = np.ascontiguousarray(
        x8_sorted.reshape(T, KT8, 2, 128).transpose(3, 1, 2, 0)
    )  # [128, kt, i, T]
    all_chunks = [c for chunks in layout for c in chunks]
    XTOT = 16 * sum(_ceil16(n) for _, n in all_chunks)
    xg_host = np.zeros((128, XTOT), dtype=FP8)
    off = 0
    for (t0, n) in all_chunks:
        n16 = _ceil16(n)
        blk = np.zeros((128, KT8, 2, n16), dtype=FP8)
        blk[:, :, :, :n] = Y[:, :, :, t0 : t0 + n]
        xg_host[:, off : off + 16 * n16] = blk.reshape(128, 16 * n16)
        off += 16 * n16

    # ---- host: raw int4 nibbles -> e4m3 (exact), [E, H, F] ----
    b = (packed_weights & 0xFF).astype(np.uint8)      # [E, F, P] byte values
    lo = (b & 0xF)                                    # even h = 2p
    hi = (b >> 4)                                     # odd  h = 2p+1
    W8 = np.empty((E, H, F), dtype=np.uint8)
    W8[:, 0::2, :] = lo.transpose(0, 2, 1)
    W8[:, 1::2, :] = hi.transpose(0, 2, 1)
    qbar = (
        lo.sum(axis=2, dtype=np.uint32) + hi.sum(axis=2, dtype=np.uint32)
    ).astype(np.float32) / np.float32(H)              # [E, F] mean_h q
    W8 = W8.astype(FP8)                               # values 0..15: exact

    # ---- build / fetch program ----
    nc = _PROGRAM_CACHE.get(chunk_ns)
    if nc is None:
        nc = _build_program(chunk_ns)
        _PROGRAM_CACHE[chunk_ns] = nc

    in_maps = []
    for c in range(NCORES):
        # [E, H, FC] -> [E, p, kt, i, FC]: h = kt*256 + i*128 + p
        wc = (
            W8[:, :, c * FC : (c + 1) * FC]
            .reshape(E, KT8, 2, 128, FC)
            .transpose(0, 3, 1, 2, 4)
        )
        in_maps.append({"xg": xg_host, "wT": np.ascontiguousarray(wc)})

    res = run_bass_kernel_spmd(nc, in_maps, list(range(NCORES)))
    LAST_RESULT = res

    # ---- gather: stack F-major slices, transpose to sorted [T, F] ----
    R_T = np.concatenate(
        [np.asarray(res.results[c]["out"]) for c in range(NCORES)], axis=0
    )  # [F, T] fp16
    R_sorted = np.ascontiguousarray(R_T.T).astype(np.float32)  # [T, F]

    # ---- host epilogue: fold scale/zp + rank-1 e4m3 mean correction ----
    #   out = s_f * R - s_f*zp_f * SX_t - s_f*qbar_f * D_t
    out_sorted = np.empty((T, F), dtype=np.float32)
    t0 = 0
    for e in range(E):
        c = int(counts[e])
        if c == 0:
            continue
        sl = slice(t0, t0 + c)
        out_sorted[sl] = (
            scales[e][None, :] * R_sorted[sl]
            - np.outer(SX[sl], scales[e] * zero_points[e])
            - np.outer(D[sl], scales[e] * qbar[e])
        )
        t0 += c

    out_full = np.empty((T, F), dtype=np.float32)
    out_full[perm] = out_sorted
    return out_full
